# revision 2
# baseline (speedup 1.0000x reference)
"""GAT message-passing (CentroidGATConv) Trainium2 Bass kernel, v5.

Strategy (8 NeuronCores, SPMD, no collectives):
  - Dst-node range sharding, npc=6400 (tile-aligned): core k owns dst nodes
    [k*6400, (k+1)*6400). dst sorted -> per-block segments core-local.
  - Phase 1 (replicated GEMM): ft = feat @ W written to two DRAM tables in
    PERMUTED layout (row = (n%128)*T + n//128) so writes are fully
    contiguous per partition (512B rows, 8KB/partition per 16-tile write).
    Per-core er for OWN dst range via mini-GEMM (featT_own @ (W@AR)) kept
    resident in SBUF. el is NOT precomputed (computed per-edge in phase 2).
  - Phase 2 (edges): per 2-block group, batch-gather src ft rows (512B) with
    two dma_gather calls (lo/hi tables, permuted int16 idx, 4 SWDGE queues),
    compute el per edge on DVE (mult by replicated attn_l + reduce), route
    er with tiny per-subchunk PE matmuls (lhsT=ohT fp8, rhs=resident er),
    w = exp(leaky_relu(el+er)), aggregate with PE matmuls
    ps[n] += oh_s.T @ [w*ft | w], divide, write permuted out.

kernel(**inputs) takes FULL inputs, shards internally, returns FULL output.
"""

import math
import os
from contextlib import ExitStack

import ml_dtypes
import numpy as np

import concourse.bacc as bacc
import concourse.bass as bass
import concourse.mybir as mybir
import concourse.tile as tile
from concourse.bass_utils import run_bass_kernel_spmd

F32 = mybir.dt.float32
BF16 = mybir.dt.bfloat16
FP8 = mybir.dt.float8e4
I16 = mybir.dt.int16
U8 = mybir.dt.uint8
FP8_ONE = 0x38            # bit pattern of 1.0 in fp8 e4m3
AF = mybir.ActivationFunctionType
OP = mybir.AluOpType

P = 128
N_CORES = 8
NEG_SLOPE = 0.2
NPC = 6400             # dst nodes per core (tile aligned)
B = 50                 # dst blocks per core
NPAD = NPC * N_CORES   # 51200
NT = NPAD // P         # 400 global node tiles
SPLIT = 32768          # node-id split for int16 gather indices
T_LO = SPLIT // P      # 256 tiles in lo table
T_HI = NT - T_LO       # 144 tiles in hi table
ROWE = 384             # gather row elements (bf16) = 768B: [ft 256|el 4|junk]
GT = 16                # GEMM tiles per group (25 groups)
GROUP = 2              # dst-node blocks per gather group
H = 4
D = 64
HD = H * D             # 256
FE = HD + H            # 260 agg matmul rhs width

LAST_RESULTS = None
_PROGRAM_CACHE = {}


class Cfg:
    def __init__(self, N, E, d_in, NSLO, NSHI):
        self.N = N
        self.E = E
        self.d_in = d_in
        self.NSLO = NSLO            # per-block lo subchunk counts [B]
        self.NSHI = NSHI
        self.KH = d_in // P
        assert d_in % P == 0

        # group layout: blocks (2g, 2g+1); subchunk order per group:
        # [b0lo | b1lo | b0hi | b1hi]
        self.groups = []
        sub_off = 0
        lo_col = 0   # int16 idx cols consumed (slots/16)
        hi_col = 0
        for g0 in range(0, B, GROUP):
            blks = list(range(g0, min(g0 + GROUP, B)))
            nslo = [NSLO[b] for b in blks]
            nshi = [NSHI[b] for b in blks]
            ns_lo_g = sum(nslo)
            ns_hi_g = sum(nshi)
            ns_g = ns_lo_g + ns_hi_g
            sec = []
            lo_s = 0
            hi_s = ns_lo_g
            for i in range(len(blks)):
                sec.append((lo_s, nslo[i], hi_s, nshi[i]))
                lo_s += nslo[i]
                hi_s += nshi[i]
            self.groups.append({
                "blks": blks, "sec": sec, "ns_lo": ns_lo_g, "ns_hi": ns_hi_g,
                "ns": ns_g, "sub_off": sub_off,
                "lo_col": lo_col, "hi_col": hi_col,
            })
            sub_off += ns_g
            lo_col += ns_lo_g * 8   # 128 slots/subchunk / 16 = 8 cols
            hi_col += ns_hi_g * 8
        self.S = sub_off
        self.L_LO = lo_col
        self.L_HI = hi_col
        self.NSG_MAX = max(g["ns"] for g in self.groups)

    def key(self):
        return (self.N, self.E, self.d_in, tuple(self.NSLO), tuple(self.NSHI))


def host_prep(feat, src, dst, W, attn_l, attn_r):
    feat = np.asarray(feat, dtype=np.float32)
    src = np.asarray(src).astype(np.int64)
    dst = np.asarray(dst).astype(np.int64)
    W = np.asarray(W, dtype=np.float32)
    attn_l = np.asarray(attn_l, dtype=np.float32)
    attn_r = np.asarray(attn_r, dtype=np.float32)

    N, d_in = feat.shape
    E = src.shape[0]

    AR = np.zeros((HD, H), np.float32)
    AL = np.zeros((HD, H), np.float32)
    for h in range(H):
        AR[h * D:(h + 1) * D, h] = attn_r[h]
        AL[h * D:(h + 1) * D, h] = attn_l[h]
    wR = (W @ AR).astype(ml_dtypes.bfloat16)            # [256, 4]
    # cols: [ft 256 | el 4]
    wc = np.concatenate([W, W @ AL], axis=1).astype(
        ml_dtypes.bfloat16)                             # [256, 260]

    core = dst // NPC
    loc = dst - core * NPC
    blk = loc // P
    key = core * B + blk
    hi_f = (src >= SPLIT).astype(np.int64)
    n_all = np.bincount(key, minlength=N_CORES * B).reshape(N_CORES, B)
    n_lo = np.bincount(key[hi_f == 0], minlength=N_CORES * B).reshape(
        N_CORES, B)
    n_hi = n_all - n_lo
    NSLO = [int(x) for x in np.ceil(n_lo.max(axis=0) / P).astype(np.int64)]
    NSHI = [int(x) for x in np.ceil(n_hi.max(axis=0) / P).astype(np.int64)]

    cfg = Cfg(N, E, d_in, NSLO, NSHI)

    # per-edge rank within (core, block, section)
    key2 = key * 2 + hi_f
    order = np.argsort(key2, kind="stable")
    counts2 = np.bincount(key2, minlength=N_CORES * B * 2)
    starts2 = np.zeros_like(counts2)
    starts2[1:] = np.cumsum(counts2)[:-1]
    rank = np.empty(E, np.int64)
    rank[order] = np.arange(E) - starts2[key2[order]]

    # static per (block, section) bases
    lo_base_j = np.zeros(B, np.int64)    # j base within the group's LO call
    hi_base_j = np.zeros(B, np.int64)
    lo_call_col = np.zeros(B, np.int64)  # idx col offset of the group's call
    hi_call_col = np.zeros(B, np.int64)
    sub_lo = np.zeros(B, np.int64)       # global subchunk index of section
    sub_hi = np.zeros(B, np.int64)
    for g in cfg.groups:
        for i, b in enumerate(g["blks"]):
            lo_s, nlo, hi_s, nhi = g["sec"][i]
            lo_base_j[b] = lo_s * P
            hi_base_j[b] = (hi_s - g["ns_lo"]) * P
            lo_call_col[b] = g["lo_col"]
            hi_call_col[b] = g["hi_col"]
            sub_lo[b] = g["sub_off"] + lo_s
            sub_hi[b] = g["sub_off"] + hi_s

    # per-edge positions
    is_hi = hi_f == 1
    j_call = np.where(is_hi, hi_base_j[blk] + rank, lo_base_j[blk] + rank)
    sub_g = np.where(is_hi, sub_hi[blk], sub_lo[blk]) + rank // P
    p_slot = rank % P
    flat_pos = np.where(is_hi, hi_call_col[blk] * 16,
                        lo_call_col[blk] * 16) + j_call
    # PERMUTED gather index: node n -> row (n%128)*T + n//128 of its table
    idx_val = np.where(is_hi,
                       (src % P) * T_HI + (src // P - T_LO),
                       (src % P) * T_LO + (src // P))

    featT = np.zeros((d_in, NPAD), ml_dtypes.bfloat16)
    featT[:, :N] = feat.T.astype(ml_dtypes.bfloat16)

    metas = []
    for c in range(N_CORES):
        m = core == c
        ilo_flat = np.zeros(cfg.L_LO * 16, np.int16)
        ihi_flat = np.zeros(cfg.L_HI * 16, np.int16)
        mlo = m & ~is_hi
        mhi = m & is_hi
        ilo_flat[flat_pos[mlo]] = idx_val[mlo].astype(np.int16)
        ihi_flat[flat_pos[mhi]] = idx_val[mhi].astype(np.int16)
        # wrap j -> [p = j%16, l = j//16], replicate to 128 partitions
        ilo_w = np.tile(ilo_flat.reshape(-1, 16).T, (8, 1))
        ihi_w = np.tile(ihi_flat.reshape(-1, 16).T, (8, 1))

        # fp8 one-hot matrices, built on host
        # oh[p, s*128+n] = 1.0 iff edge at slot (p, s) routes to local node n
        # ohT[n, s*128+p] = same, node-partitioned (for er routing)
        rel = (loc[m] % P).astype(np.int64)
        sg = sub_g[m]
        pp = p_slot[m]
        oh_u8 = np.zeros(P * cfg.S * P, np.uint8)
        oh_u8[(pp * cfg.S + sg) * P + rel] = FP8_ONE
        oh_u8 = oh_u8.reshape(P, cfg.S * P)
        ohT_u8 = np.zeros(P * cfg.S * P, np.uint8)
        ohT_u8[(rel * cfg.S + sg) * P + pp] = FP8_ONE
        ohT_u8 = ohT_u8.reshape(P, cfg.S * P)

        metas.append({
            "ilo": np.ascontiguousarray(ilo_w),
            "ihi": np.ascontiguousarray(ihi_w),
            "oh": oh_u8,
            "ohT": ohT_u8,
            "featT_own": np.ascontiguousarray(
                featT[:, c * NPC:(c + 1) * NPC]),
        })

    return cfg, featT, wc, wR, metas


def build_program(cfg: Cfg):
    nc = bacc.Bacc("TRN2", target_bir_lowering=False, debug=False,
                   num_devices=N_CORES, num_swdge_queues=4)

    featT = nc.dram_tensor("featT", [cfg.d_in, NPAD], BF16,
                           kind="ExternalInput").ap()
    featT_own = nc.dram_tensor("featT_own", [cfg.d_in, NPC], BF16,
                               kind="ExternalInput").ap()
    wc = nc.dram_tensor("wc", [cfg.d_in, FE], BF16,
                        kind="ExternalInput").ap()
    wR = nc.dram_tensor("wR", [cfg.d_in, H], BF16,
                        kind="ExternalInput").ap()
    ilo = nc.dram_tensor("ilo", [P, cfg.L_LO], I16, kind="ExternalInput").ap()
    ihi = nc.dram_tensor("ihi", [P, cfg.L_HI], I16, kind="ExternalInput").ap()
    oh_d = nc.dram_tensor("oh", [P, cfg.S * P], U8, kind="ExternalInput").ap()
    ohT_d = nc.dram_tensor("ohT", [P, cfg.S * P], U8,
                           kind="ExternalInput").ap()
    out = nc.dram_tensor("out_perm", [P, B, HD], F32,
                         kind="ExternalOutput").ap()
    ftel_lo = nc.dram_tensor("ftel_lo", [T_LO * P, ROWE], BF16).ap()
    ftel_hi = nc.dram_tensor("ftel_hi", [T_HI * P, ROWE], BF16).ap()

    dbg_phase = int(os.environ.get("DBG_PHASE", "0"))
    with tile.TileContext(nc) as tc, ExitStack() as ctx:
        st = _setup_phase(ctx, tc, cfg, featT_own, wc, wR, ilo, ihi)
        if dbg_phase in (0, 1):
            _gemm_phase(ctx, tc, cfg, featT, st, ftel_lo, ftel_hi)
        if dbg_phase in (0, 2):
            _edge_phase(ctx, tc, cfg, st, ftel_lo, ftel_hi, oh_d, ohT_d, out)
    nc.compile()
    return nc


def _setup_phase(ctx, tc, cfg, featT_own, wc, wR, ilo, ihi):
    """Load resident tiles + per-core er mini-GEMM (er = feat_own @ W@AR)."""
    nc = tc.nc
    mpool = ctx.enter_context(tc.tile_pool(name="resident", bufs=1))
    fpool = ctx.enter_context(tc.tile_pool(name="fown", bufs=1))
    epool = ctx.enter_context(tc.tile_pool(name="er_ps", bufs=1,
                                           space="PSUM"))

    w_sb = mpool.tile([P, cfg.KH, FE], BF16, tag="w_sb")
    for k in range(cfg.KH):
        nc.sync.dma_start(out=w_sb[:, k, :], in_=wc[k * P:(k + 1) * P, :])
    wr_sb = mpool.tile([P, cfg.KH, H], BF16, tag="wr_sb")
    for k in range(cfg.KH):
        nc.sync.dma_start(out=wr_sb[:, k, :], in_=wR[k * P:(k + 1) * P, :])
    ilo_sb = mpool.tile([P, cfg.L_LO], I16, tag="ilo_sb")
    nc.sync.dma_start(out=ilo_sb[:, :], in_=ilo[:, :])
    ihi_sb = mpool.tile([P, cfg.L_HI], I16, tag="ihi_sb")
    nc.sync.dma_start(out=ihi_sb[:, :], in_=ihi[:, :])
    er_loc = mpool.tile([P, B, H], BF16, tag="er_loc")

    # er mini-GEMM over own dst range, 2 passes of 25 tiles
    HB = B // 2
    fown = fpool.tile([P, cfg.KH, HB * P], BF16, tag="fown")
    for half in range(2):
        c0 = half * HB * P
        for k in range(cfg.KH):
            nc.sync.dma_start(out=fown[:, k, :],
                              in_=featT_own[k * P:(k + 1) * P,
                                            c0:c0 + HB * P])
        for t in range(HB):
            b = half * HB + t
            ps = epool.tile([P, H], F32, tag="erps")
            for k in range(cfg.KH):
                nc.tensor.matmul(out=ps[:, :],
                                 lhsT=fown[:, k, t * P:(t + 1) * P],
                                 rhs=wr_sb[:, k, :],
                                 start=(k == 0), stop=(k == cfg.KH - 1))
            if t % 2 == 0:
                nc.scalar.copy(out=er_loc[:, b, :], in_=ps[:, :])
            else:
                nc.vector.tensor_copy(out=er_loc[:, b, :], in_=ps[:, :])

    return {"w_sb": w_sb, "ilo_sb": ilo_sb,
            "ihi_sb": ihi_sb, "er_loc": er_loc}


def _gemm_phase(ctx, tc, cfg, featT, st, ftel_lo, ftel_hi):
    nc = tc.nc
    w_sb = st["w_sb"]
    lo_v = ftel_lo.rearrange("(p t) c -> p t c", p=P)   # [128, T_LO, 384]
    hi_v = ftel_hi.rearrange("(p t) c -> p t c", p=P)   # [128, T_HI, 384]

    lpool = ctx.enter_context(tc.tile_pool(name="featT_stage", bufs=2))
    spool = ctx.enter_context(tc.tile_pool(name="ft_stage", bufs=2))
    pspool = ctx.enter_context(tc.tile_pool(name="gemm_ps", bufs=2,
                                            space="PSUM"))

    ngroups = NT // GT           # 25
    for g in range(ngroups):
        g0 = g * GT
        c0 = g0 * P
        cols = GT * P
        stage_in = lpool.tile([P, cfg.KH, GT * P], BF16, tag="featT_stage")
        for k in range(cfg.KH):
            nc.sync.dma_start(out=stage_in[:, k, :],
                              in_=featT[k * P:(k + 1) * P, c0:c0 + cols])
        stage_out = spool.tile([P, GT, ROWE], BF16, tag="ft_stage")
        for t in range(GT):
            ps = pspool.tile([P, FE], F32)
            for k in range(cfg.KH):
                nc.tensor.matmul(out=ps[:, :],
                                 lhsT=stage_in[:, k, t * P:(t + 1) * P],
                                 rhs=w_sb[:, k, :],
                                 start=(k == 0), stop=(k == cfg.KH - 1))
            if t % 2 == 0:
                nc.scalar.copy(out=stage_out[:, t, 0:FE], in_=ps[:, :])
            else:
                nc.vector.tensor_copy(out=stage_out[:, t, 0:FE], in_=ps[:, :])
        if g0 >= T_LO:
            dst = hi_v[:, g0 - T_LO:g0 - T_LO + GT, :]
        else:
            dst = lo_v[:, g0:g0 + GT, :]
        nc.sync.dma_start(out=dst, in_=stage_out[:, :, :])


def _edge_phase(ctx, tc, cfg, st, ftel_lo, ftel_hi, oh_d, ohT_d, out):
    nc = tc.nc
    ilo_sb = st["ilo_sb"]
    ihi_sb = st["ihi_sb"]
    er_loc = st["er_loc"]
    NSG = cfg.NSG_MAX

    gpool = ctx.enter_context(tc.tile_pool(name="gather", bufs=4))
    opool = ctx.enter_context(tc.tile_pool(name="onehot", bufs=2))
    otpool = ctx.enter_context(tc.tile_pool(name="onehotT", bufs=2))
    spool = ctx.enter_context(tc.tile_pool(name="score", bufs=2))
    dpool = ctx.enter_context(tc.tile_pool(name="denom", bufs=2))
    outpool = ctx.enter_context(tc.tile_pool(name="outsb", bufs=2))
    aggps = ctx.enter_context(tc.tile_pool(name="agg_ps", bufs=3,
                                           space="PSUM"))
    erps = ctx.enter_context(tc.tile_pool(name="erroute_ps", bufs=2,
                                          space="PSUM"))

    dbg_groups = int(os.environ.get("DBG_GROUPS", "0"))
    groups = cfg.groups[:dbg_groups] if dbg_groups else cfg.groups
    for gi, g in enumerate(groups):
        ns = g["ns"]
        ns_lo = g["ns_lo"]
        ns_hi = g["ns_hi"]
        sub0 = g["sub_off"]

        gt = gpool.tile([P, NSG, ROWE], BF16, tag="g")
        if ns_lo:
            nc.gpsimd.dma_gather(
                out_ap=gt[:, 0:ns_lo, :], in_ap=ftel_lo[:, :],
                idxs_ap=ilo_sb[:, g["lo_col"]:g["lo_col"] + ns_lo * 8],
                num_idxs=ns_lo * P, num_idxs_reg=ns_lo * P, elem_size=ROWE,
                single_packet=False, queue_num=gi % 4)
        if ns_hi:
            nc.gpsimd.dma_gather(
                out_ap=gt[:, ns_lo:ns, :], in_ap=ftel_hi[:, :],
                idxs_ap=ihi_sb[:, g["hi_col"]:g["hi_col"] + ns_hi * 8],
                num_idxs=ns_hi * P, num_idxs_reg=ns_hi * P, elem_size=ROWE,
                single_packet=False, queue_num=(gi + 2) % 4)

        oh = opool.tile([P, NSG * P], U8, tag="oh")
        nc.sync.dma_start(out=oh[:, 0:ns * P],
                          in_=oh_d[:, sub0 * P:(sub0 + ns) * P])
        ohT = otpool.tile([P, NSG * P], U8, tag="ohT")
        nc.sync.dma_start(out=ohT[:, 0:ns * P],
                          in_=ohT_d[:, sub0 * P:(sub0 + ns) * P])

        # er routing: ps_er[p, s*H:(s+1)*H] = ohT_s^T @ er_loc[:, b, :]
        ps_er = erps.tile([P, NSG * H], F32)
        for i, b in enumerate(g["blks"]):
            lo_s, nlo, hi_s, nhi = g["sec"][i]
            for s in list(range(lo_s, lo_s + nlo)) + \
                     list(range(hi_s, hi_s + nhi)):
                nc.tensor.matmul(out=ps_er[:, s * H:(s + 1) * H],
                                 lhsT=ohT[:, s * P:(s + 1) * P].bitcast(FP8),
                                 rhs=er_loc[:, b, :], start=True, stop=True)

        # el rides in the gathered row at cols [256:260]
        sc = spool.tile([P, NSG, H], F32, tag="sc")
        nc.vector.tensor_tensor(
            out=sc[:, 0:ns, :], in0=gt[:, 0:ns, HD:FE],
            in1=ps_er[:, 0:ns * H].rearrange("p (s h) -> p s h", h=H),
            op=OP.add)
        lk = spool.tile([P, NSG, H], F32, tag="lk")
        nc.vector.scalar_tensor_tensor(out=lk[:, 0:ns, :], in0=sc[:, 0:ns, :],
                                       scalar=NEG_SLOPE, in1=sc[:, 0:ns, :],
                                       op0=OP.mult, op1=OP.max)
        w = spool.tile([P, NSG, H], BF16, tag="w")
        nc.scalar.activation(out=w[:, 0:ns, :], in_=lk[:, 0:ns, :],
                             func=AF.Exp)

        # rhs = [w*ft | w], built in place in gt (el cols already consumed)
        rhs = gt
        nc.vector.tensor_tensor(
            out=rhs[:, 0:ns, 0:HD].rearrange("p s (h d) -> p s h d", h=H),
            in0=gt[:, 0:ns, 0:HD].rearrange("p s (h d) -> p s h d", h=H),
            in1=w[:, 0:ns, :].to_broadcast([P, ns, H, D]),
            op=OP.mult)
        nc.scalar.copy(out=rhs[:, 0:ns, HD:FE], in_=w[:, 0:ns, :])

        outsb = outpool.tile([P, GROUP, HD], F32, tag="out")
        for i, b in enumerate(g["blks"]):
            lo_s, nlo, hi_s, nhi = g["sec"][i]
            subs = list(range(lo_s, lo_s + nlo)) + \
                   list(range(hi_s, hi_s + nhi))
            if not subs:
                nc.vector.memset(outsb[:, i, :], 0.0)
                continue
            ps = aggps.tile([P, FE], F32, tag="agg")
            for j, s in enumerate(subs):
                nc.tensor.matmul(out=ps[:, :],
                                 lhsT=oh[:, s * P:(s + 1) * P].bitcast(FP8),
                                 rhs=rhs[:, s, 0:FE],
                                 start=(j == 0), stop=(j == len(subs) - 1))
            den = dpool.tile([P, H], F32, tag="den")
            nc.vector.tensor_scalar_add(out=den[:, :], in0=ps[:, HD:FE],
                                        scalar1=1e-30)
            recip = dpool.tile([P, H], F32, tag="recip")
            nc.vector.reciprocal(out=recip[:, :], in_=den[:, :])
            nc.vector.tensor_tensor(
                out=outsb[:, i, :].rearrange("p (h d) -> p h d", h=H),
                in0=ps[:, 0:HD].rearrange("p (h d) -> p h d", h=H),
                in1=recip[:, :].to_broadcast([P, H, D]),
                op=OP.mult)
        nb = len(g["blks"])
        nc.sync.dma_start(out=out[:, g["blks"][0]:g["blks"][0] + nb, :],
                          in_=outsb[:, 0:nb, :])


def kernel(feat, src, dst, W, attn_l, attn_r):
    global LAST_RESULTS
    cfg, featT, wc, wR, metas = host_prep(
        feat, src, dst, W, attn_l, attn_r)

    nc = _PROGRAM_CACHE.get(cfg.key())
    if nc is None:
        nc = build_program(cfg)
        _PROGRAM_CACHE[cfg.key()] = nc

    in_maps = []
    for c in range(N_CORES):
        m = {"featT": featT, "wc": wc, "wR": wR}
        m.update(metas[c])
        in_maps.append(m)

    dbg_cores = int(os.environ.get("DBG_CORES", str(N_CORES)))
    res = run_bass_kernel_spmd(nc, in_maps[:dbg_cores],
                               list(range(dbg_cores)))
    LAST_RESULTS = res

    N = cfg.N
    out_full = np.zeros((N, HD), np.float32)
    for c in range(dbg_cores):
        lo = c * NPC
        hi = min(lo + NPC, N)
        if hi > lo:
            op = res.results[c]["out_perm"]          # [128, B, 256]
            flat = op.transpose(1, 0, 2).reshape(NPC, HD)
            out_full[lo:hi] = flat[:hi - lo]
    return out_full.reshape(N, H, D)
